# revision 12
# baseline (speedup 1.0000x reference)
"""Multi-head attention (B=2, S=2048, D=1024, H=16, d_k=64) on 8 TRN2 NeuronCores.

Sharding: batch x head-groups. Core c handles batch b = c // 4 and heads
[4*(c%4), 4*(c%4)+4), i.e. a 256-wide slice of the model dim. Host sums the
4 partial y's per batch and adds bo.

Per-core kernel, organized so every matmul instruction's cost (= its
moving/free size) does useful work, and so the Scalar engine (softmax exp,
~133us busy, the co-critical engine next to the PE's ~146us) starts early
and never starves:
  - scores S^T = K Q^T per head-pair: two (64x128)-stationary matmuls per
    kt tile, free dim 512 (f32r, full rate),
  - attention out in [query, d_k] orientation: stationary = exp-score tile
    pt [128 keys, 128 queries], moving = ones-AUGMENTED V [128 keys, 65]
    (64 v-columns + a ones column), accumulated over the 16 key tiles in
    PSUM (one accumulation group per bank: start only on the first write,
    stop on the last - PSUM zero regions are 2KB). Free size is 65 instead
    of 512, and column 64 accumulates the softmax denominator for free,
  - normalization is partition-aligned: DVE reciprocal of the denominator
    column + tensor_scalar multiply per [128, 64] block -> attn [q, dk]
    bf16 in SBUF,
  - a PE transpose (identity matmul, 128 rows each) flips attn back to
    [dk, token] for the O-projection; 2 heads per transpose,
  - phase 1 fuses K/V/Q(t4=0) projections with qb0's full score+exp chain
    (kept lean so the first exp lands ~10us in); Q(t4=1..3) projections,
    next-qb scores, transposes and O-projection chunks all flow through one
    ordered work queue, popped as filler BEFORE the exp-gated AV matmuls
    (PE queue is in-order, so filler must precede the stall),
  - transpose + O-proj PSUM tiles share one 2-deep tag ring so consecutive
    O-proj chunks don't serialize on the DVE drain of the previous chunk.

PSUM budget (8 banks): sct ring 4 | avq 2 (one bank per head) | shared
transpose/O-proj ring 2. Phase 1 swaps the last ring for a 2-bank
projection ring (LIFO pool order). Matmuls: f32r for scores (full rate at
free dim >= 256), bf16 elsewhere; f32 accumulation throughout.
"""

import numpy as np

B, S, D = 2, 2048, 1024
H, DK = 16, 64
NCORES = 8
DS = 256            # model-dim slice per core (4 heads x 64)
P = 128
DKA = DK + 1        # v columns + softmax-denominator ones column

_cache = {}


def _build(repeat=1):
    import concourse.mybir as mybir
    import concourse.tile as tile
    from concourse import bacc

    f32 = mybir.dt.float32
    f32r = mybir.dt.float32r
    bf16 = mybir.dt.bfloat16
    Exp = mybir.ActivationFunctionType.Exp
    add = mybir.AluOpType.add
    mult = mybir.AluOpType.mult

    nc = bacc.Bacc("TRN2", target_bir_lowering=False, debug=False,
                   num_devices=NCORES)

    xq_d = nc.dram_tensor("xq", [S, D], bf16, kind="ExternalInput")
    xk_d = nc.dram_tensor("xk", [S, D], bf16, kind="ExternalInput")
    xv_d = nc.dram_tensor("xv", [S, D], bf16, kind="ExternalInput")
    wqT_d = nc.dram_tensor("wqT", [D, DS], bf16, kind="ExternalInput")
    wkT_d = nc.dram_tensor("wkT", [D, DS], bf16, kind="ExternalInput")
    wvT_d = nc.dram_tensor("wvT", [D, DS], bf16, kind="ExternalInput")
    woT_d = nc.dram_tensor("woT", [DS, D], bf16, kind="ExternalInput")
    bq_d = nc.dram_tensor("bq", [2, P, 1], f32, kind="ExternalInput")
    bk_d = nc.dram_tensor("bk", [2, P, 1], f32, kind="ExternalInput")
    bvr_d = nc.dram_tensor("bvr", [P, DS], bf16, kind="ExternalInput")
    id_d = nc.dram_tensor("ident", [P, P], bf16, kind="ExternalInput")
    y_d = nc.dram_tensor("y", [S, D], f32, kind="ExternalOutput")

    with tile.TileContext(nc) as tc:
        with (
            tc.tile_pool(name="persist", bufs=1) as pp,
            tc.tile_pool(name="xT", bufs=4) as xtp,
            tc.tile_pool(name="pt", bufs=48) as ptp,
            tc.tile_pool(name="attn", bufs=4) as asp,
            tc.tile_pool(name="small", bufs=2) as smp,
            tc.tile_pool(name="ysb", bufs=2) as yp,
        ):
            # ---- constants / weights ----
            wq_bf = pp.tile([P, 8, DS], bf16)
            wk_bf = pp.tile([P, 8, DS], bf16)
            wv_bf = pp.tile([P, 8, DS], bf16)
            wo_bf = pp.tile([P, 2, D], bf16)
            bq_sb = pp.tile([P, 2, 1], f32)
            bk_sb = pp.tile([P, 2, 1], f32)
            bv_sb = pp.tile([P, DS], bf16)
            id_sb = pp.tile([P, P], bf16)

            # ---- persistent activations ----
            QT = pp.tile([P, 2, S], f32r)      # [dk-in-pair, head-pair, token]
            KT = pp.tile([P, 2, S], f32r)
            V = pp.tile([P, 16, 4, DKA], bf16)  # [key-in-tile, kt, head, dk+1]
            attnT = pp.tile([P, 2, S], bf16)   # [dk-in-pair, head-pair, token]
            # softmax-denominator ones column, preset once
            nc.vector.memset(V[:, :, :, DK:DKA], 1.0)

            for _rep in range(repeat):
                sc_ctx = tc.tile_pool(name="sc_ps", bufs=2, space="PSUM")
                scp = sc_ctx.__enter__()
                tr_ctx = tc.tile_pool(name="tr_ps", bufs=2, space="PSUM")
                trp = tr_ctx.__enter__()

                # K-projection inputs first on the DMA queue
                nc.scalar.dma_start(
                    wk_bf[:], wkT_d.ap().rearrange("(c p) d -> p c d", p=P))

                warm = pp.tile([P, 128], bf16, name="warm", tag="warm") \
                    if _rep == 0 else warm
                if _rep == 0:
                    nc.vector.memset(warm[:], 0.0)
                # ~5us of dependency-free matmuls: keeps the PE busy (and
                # its clock-gate warm) through the startup DMA fill, so the
                # first projection matmuls run at full clock
                for _w in range(48):
                    wps = trp.tile([P, 512], f32, tag="pj", name="wps")
                    nc.tensor.matmul(wps[:, 0:128], warm[:], warm[:],
                                     start=True, stop=True)

                pts = {}

                def emit_scores(qb, kt):
                    qs = slice(qb * 512, (qb + 1) * 512)
                    for hp in range(2):
                        sct = scp.tile([P, 2, 512], f32, tag="sct")
                        for hh in range(2):
                            hb = 64 * hh
                            nc.tensor.matmul(
                                sct[:, hh, :],
                                KT[hb:hb + 64, hp, kt * P:(kt + 1) * P],
                                QT[hb:hb + 64, hp, qs],
                                start=True, stop=True)
                        pt = ptp.tile([P, 2, 512], bf16, tag="pt")
                        nc.scalar.activation(pt[:], sct[:], Exp, scale=0.125)
                        pts[(qb, kt, hp)] = pt

                def load_xT(x_d, t4):
                    xT = xtp.tile([P, 8, 512], bf16, tag="xT")
                    for i in range(4):
                        tb = 4 * t4 + i
                        nc.sync.dma_start_transpose(
                            xT[:, :, i * P:(i + 1) * P],
                            x_d.ap()[tb * P:(tb + 1) * P, :])
                    return xT

                def emit_proj(kind, t4, xT, hp, fine=False, pool=None):
                    w = wk_bf if kind == "k" else wq_bf
                    bias = bk_sb if kind == "k" else bq_sb
                    out = KT if kind == "k" else QT
                    # fine=True: 128-token chunks so the first matmuls start
                    # after transpose 0 lands, not after all four
                    chunks = [slice(128 * i, 128 * (i + 1)) for i in range(4)] \
                        if fine else [slice(0, 512)]
                    if pool is None:
                        ps = trp.tile([P, 512], f32, tag="pj", name="ps")
                    else:
                        ps = pool.tile([P, 512], f32, tag="tpy", name="ps")
                    ob = t4 * 512
                    for cs in chunks:
                        for ch in range(8):
                            nc.tensor.matmul(
                                ps[:, cs], w[:, ch, hp * P:(hp + 1) * P],
                                xT[:, ch, cs],
                                start=(ch == 0), stop=(ch == 7))
                        if fine:
                            nc.vector.tensor_scalar(
                                out[:, hp, ob + cs.start:ob + cs.stop],
                                ps[:, cs], bias[:, hp, :], None, op0=add)
                    if not fine:
                        nc.vector.tensor_scalar(
                            out[:, hp, ob:ob + 512], ps[:],
                            bias[:, hp, :], None, op0=add)

                def emit_v_half(xT, t4, half):
                    pv = trp.tile([P, 512], f32, tag="pj")
                    pvv = pv[:].rearrange("p (t d) -> p t d", t=2)
                    for j in range(2):
                        ti = 2 * half + j
                        for ch in range(8):
                            nc.tensor.matmul(
                                pvv[:, j, :],
                                xT[:, ch, ti * P:(ti + 1) * P],
                                wv_bf[:, ch, :],
                                start=(ch == 0), stop=(ch == 7))
                    for j in range(2):
                        tb = 4 * t4 + 2 * half + j
                        nc.vector.tensor_add(
                            V[:, tb, :, 0:DK],
                            pvv[:, j, :].rearrange("p (h d) -> p h d", h=4),
                            bv_sb[:].rearrange("p (h d) -> p h d", h=4))

                # ---- phase 1: K/V/Q0 projections fused with qb0 scores;
                # K is chunk-interleaved so each score tile emits right
                # after its 128-token K chunk and the exp stream never gaps
                xq_tiles = {}
                for t4 in range(4):
                    xTk = load_xT(xk_d, t4)
                    if t4 == 0:
                        xq_tiles[0] = load_xT(xq_d, 0)
                        nc.scalar.dma_start(bk_sb[:, 0, :], bk_d.ap()[0])
                        nc.scalar.dma_start(bk_sb[:, 1, :], bk_d.ap()[1])
                        nc.scalar.dma_start(
                            wq_bf[:],
                            wqT_d.ap().rearrange("(c p) d -> p c d", p=P))
                        nc.scalar.dma_start(bq_sb[:, 0, :], bq_d.ap()[0])
                        nc.scalar.dma_start(bq_sb[:, 1, :], bq_d.ap()[1])
                        emit_proj("k", 0, xTk, 0, fine=True)
                        emit_proj("k", 0, xTk, 1, fine=True)
                        emit_proj("q", 0, xq_tiles[0], 0, fine=True)
                        emit_proj("q", 0, xq_tiles[0], 1, fine=True)
                        # deferred constant loads, off the startup DMA path
                        nc.scalar.dma_start(
                            wv_bf[:],
                            wvT_d.ap().rearrange("(c p) d -> p c d", p=P))
                        nc.scalar.dma_start(bv_sb[:], bvr_d.ap())
                        nc.scalar.dma_start(id_sb[:], id_d.ap())
                        nc.scalar.dma_start(
                            wo_bf[:],
                            woT_d.ap().rearrange("(c p) d -> p c d", p=P))
                        xTv = load_xT(xv_d, t4)
                        emit_scores(0, 0)
                        emit_scores(0, 1)
                        emit_v_half(xTv, t4, 0)
                        emit_scores(0, 2)
                        emit_scores(0, 3)
                        emit_v_half(xTv, t4, 1)
                        continue
                    xTv = load_xT(xv_d, t4)
                    kps = [trp.tile([P, 512], f32, tag="pj", name=f"kps{_h}")
                           for _h in range(2)]
                    for i in range(4):
                        cs = slice(128 * i, 128 * (i + 1))
                        for hp in range(2):
                            for ch in range(8):
                                nc.tensor.matmul(
                                    kps[hp][:, cs],
                                    wk_bf[:, ch, hp * P:(hp + 1) * P],
                                    xTk[:, ch, cs],
                                    start=(ch == 0), stop=(ch == 7))
                            nc.vector.tensor_scalar(
                                KT[:, hp, t4 * 512 + cs.start:
                                   t4 * 512 + cs.stop],
                                kps[hp][:, cs], bk_sb[:, hp, :],
                                None, op0=add)
                        emit_scores(0, 4 * t4 + i)
                        if i == 1:
                            emit_v_half(xTv, t4, 0)
                        if i == 3:
                            emit_v_half(xTv, t4, 1)

                # phase 1 projection PSUM ring -> AV + transpose/O-proj rings
                tr_ctx.__exit__(None, None, None)
                av_ctx = tc.tile_pool(name="av_ps", bufs=2, space="PSUM")
                avp = av_ctx.__enter__()
                ty_ctx = tc.tile_pool(name="ty_ps", bufs=2, space="PSUM")
                typ = ty_ctx.__enter__()

                # ---- attention (hp-outer) + work-queue filler ----
                work = []
                attn_sbs = {}
                y_sbs = {}

                def emit_T(qb, hp):
                    # PE transposes attn [q, dk] -> attnT [dk, q]; 2 heads
                    # stack per instruction via the [q, (hh dk)] input view
                    tpt = typ.tile([P, 4, P], bf16, tag="tpy", name="tpt")
                    a_sb = attn_sbs.pop((qb, hp))
                    for qt in range(4):
                        nc.tensor.transpose(tpt[:, qt, :],
                                            a_sb[:, qt, :, :], id_sb[:])
                    nc.vector.tensor_copy(
                        attnT[:, hp, qb * 512:(qb + 1) * 512]
                        .rearrange("p (t q) -> p t q", t=4),
                        tpt[:, :, :])

                tail = [False]

                def emit_y_half(tt, nb):
                    if tt not in y_sbs:
                        y_sbs[tt] = yp.tile([P, D], f32, name="y_sb", tag="y")
                    y_sb = y_sbs[tt]
                    py = typ.tile([P, 512], f32, tag="tpy", name="py")
                    for hpc in range(2):
                        nc.tensor.matmul(
                            py[:],
                            attnT[:, hpc, tt * P:(tt + 1) * P],
                            wo_bf[:, hpc, nb * 512:(nb + 1) * 512],
                            start=(hpc == 0), stop=(hpc == 1))
                    if tail[0] and (tt + nb) % 2 == 0:
                        nc.scalar.copy(y_sb[:, nb * 512:(nb + 1) * 512],
                                       py[:])
                    else:
                        nc.vector.tensor_copy(
                            y_sb[:, nb * 512:(nb + 1) * 512], py[:])
                    if nb == 1:
                        nc.scalar.dma_start(y_d.ap()[tt * P:(tt + 1) * P, :],
                                          y_sb[:])
                        del y_sbs[tt]

                def emit_work(n):
                    for _ in range(n):
                        if not work:
                            return
                        kind, *a = work.pop(0)
                        if kind == "S":
                            emit_scores(*a)
                        elif kind == "XQ":
                            xq_tiles[a[0]] = load_xT(xq_d, a[0])
                        elif kind == "Q":
                            emit_proj("q", a[0], xq_tiles[a[0]], a[1],
                                      pool=typ)
                        elif kind == "T":
                            emit_T(*a)
                        else:
                            emit_y_half(*a)

                xq_tiles[1] = load_xT(xq_d, 1)
                for qb in range(4):
                    if qb < 3:
                        # next-qb Q projection + scores feed the queue;
                        # ordering keeps Q(t4) ahead of its S(qb, *) users
                        if qb == 0:
                            work.append(("Q", 1, 0))
                            work.append(("Q", 1, 1))
                            work.append(("XQ", 2))
                        for kt in range(16):
                            work.append(("S", qb + 1, kt))
                        if qb == 0:
                            work.append(("XQ", 3))
                            work.append(("Q", 2, 0))
                            work.append(("Q", 2, 1))
                        if qb == 1:
                            work.append(("Q", 3, 0))
                            work.append(("Q", 3, 1))
                    for hp in range(2):
                        avq = [avp.tile([P, 4, DKA], f32, tag="avq",
                                        name=f"avq{_h}")
                               for _h in range(2)]
                        for kt in range(16):
                            # PE queue is in-order: pop independent filler
                            # BEFORE the exp-gated AV matmuls so it runs
                            # during the wait
                            if kt % 2 == 0:
                                emit_work(1)
                            if (hp * 16 + kt) % 4 == 0:
                                emit_work(1)
                            ptm = pts[(qb, kt, hp)]
                            for hh in range(2):
                                h = 2 * hp + hh
                                for qt in range(4):
                                    # one PSUM accumulation group per bank:
                                    # start marks the whole 2KB zero region,
                                    # each qt's first write then zero-fills
                                    nc.tensor.matmul(
                                        avq[hh][:, qt, :],
                                        ptm[:, hh, qt * P:(qt + 1) * P],
                                        V[:, kt, h, :],
                                        start=(kt == 0 and qt == 0),
                                        stop=(kt == 15 and qt == 3))
                            del pts[(qb, kt, hp)]
                        # boundary filler between the final AV and the DVE
                        # normalization chain
                        emit_work(2)
                        # partition-aligned normalization: reciprocal of the
                        # accumulated denominator column, then one
                        # tensor_scalar multiply per [128, 64] block
                        a_sb = asp.tile([P, 4, 2, DK], bf16, tag="attn")
                        attn_sbs[(qb, hp)] = a_sb
                        for hh in range(2):
                            rec = smp.tile([P, 4, 1], f32, tag="rec")
                            nc.vector.reciprocal(rec[:],
                                                 avq[hh][:, :, DK:DKA])
                            for qt in range(4):
                                nc.vector.tensor_scalar(
                                    a_sb[:, qt, hh, :],
                                    avq[hh][:, qt, 0:DK],
                                    rec[:, qt, :], None, op0=mult)
                        work.append(("T", qb, hp))
                        if hp == 1:
                            for tt in range(4 * qb, 4 * qb + 4):
                                for nb in range(2):
                                    work.append(("Y", tt, nb))
                tail[0] = True
                emit_work(len(work))

                for ctx in (ty_ctx, av_ctx, sc_ctx):
                    ctx.__exit__(None, None, None)

    nc.compile()
    return nc


def _shard(query, key, value, Wq, bq, Wk, bk, Wv, bv, Wo, bo):
    import ml_dtypes
    f = np.float32
    bf = ml_dtypes.bfloat16
    q = np.asarray(query, dtype=f).astype(bf)
    k = np.asarray(key, dtype=f).astype(bf)
    v = np.asarray(value, dtype=f).astype(bf)
    ident = np.eye(P, dtype=bf)
    in_maps = []
    for c in range(NCORES):
        b, hg = c // 4, c % 4
        ds = DS * hg
        bv_r = np.broadcast_to(
            np.asarray(bv, f)[ds:ds + DS].astype(bf)[None, :], (P, DS))
        in_maps.append({
            "xq": np.ascontiguousarray(q[b]),
            "xk": np.ascontiguousarray(k[b]),
            "xv": np.ascontiguousarray(v[b]),
            "wqT": np.ascontiguousarray(np.asarray(Wq, f)[ds:ds + DS, :].T.astype(bf)),
            "wkT": np.ascontiguousarray(np.asarray(Wk, f)[ds:ds + DS, :].T.astype(bf)),
            "wvT": np.ascontiguousarray(np.asarray(Wv, f)[ds:ds + DS, :].T.astype(bf)),
            "woT": np.ascontiguousarray(np.asarray(Wo, f)[:, ds:ds + DS].T.astype(bf)),
            "bq": np.asarray(bq, f)[ds:ds + DS].reshape(2, P, 1),
            "bk": np.asarray(bk, f)[ds:ds + DS].reshape(2, P, 1),
            "bvr": np.ascontiguousarray(bv_r),
            "ident": ident,
        })
    return in_maps


def _unshard(results, bo):
    y = np.zeros((B, S, D), dtype=np.float64)
    for c in range(NCORES):
        y[c // 4] += results[c]["y"].astype(np.float64)
    y += np.asarray(bo, np.float64)
    return y.astype(np.float32)


def kernel(query, key, value, Wq, bq, Wk, bk, Wv, bv, Wo, bo):
    from concourse.bass_utils import run_bass_kernel_spmd

    if "nc" not in _cache:
        _cache["nc"] = _build()
    nc = _cache["nc"]
    in_maps = _shard(query, key, value, Wq, bq, Wk, bk, Wv, bv, Wo, bo)
    res = run_bass_kernel_spmd(nc, in_maps, core_ids=list(range(NCORES)))
    return _unshard(res.results, bo)


# revision 13
# speedup vs baseline: 1.0006x; 1.0006x over previous
"""Multi-head attention (B=2, S=2048, D=1024, H=16, d_k=64) on 8 TRN2 NeuronCores.

Sharding: batch x head-groups. Core c handles batch b = c // 4 and heads
[4*(c%4), 4*(c%4)+4), i.e. a 256-wide slice of the model dim. Host sums the
4 partial y's per batch and adds bo.

Per-core kernel, organized so every matmul instruction's cost (= its
moving/free size) does useful work, and so the Scalar engine (softmax exp,
~133us busy, the co-critical engine next to the PE's ~146us) starts early
and never starves:
  - scores S^T = K Q^T per head-pair: two (64x128)-stationary matmuls per
    kt tile, free dim 512 (f32r, full rate),
  - attention out in [query, d_k] orientation: stationary = exp-score tile
    pt [128 keys, 128 queries], moving = ones-AUGMENTED V [128 keys, 65]
    (64 v-columns + a ones column), accumulated over the 16 key tiles in
    PSUM (one accumulation group per bank: start only on the first write,
    stop on the last - PSUM zero regions are 2KB). Free size is 65 instead
    of 512, and column 64 accumulates the softmax denominator for free,
  - normalization is partition-aligned: DVE reciprocal of the denominator
    column + tensor_scalar multiply per [128, 64] block -> attn [q, dk]
    bf16 in SBUF,
  - a PE transpose (identity matmul, 128 rows each) flips attn back to
    [dk, token] for the O-projection; 2 heads per transpose,
  - phase 1 fuses K/V/Q(t4=0) projections with qb0's full score+exp chain
    (kept lean so the first exp lands ~10us in); Q(t4=1..3) projections,
    next-qb scores, transposes and O-projection chunks all flow through one
    ordered work queue, popped as filler BEFORE the exp-gated AV matmuls
    (PE queue is in-order, so filler must precede the stall),
  - transpose + O-proj PSUM tiles share one 2-deep tag ring so consecutive
    O-proj chunks don't serialize on the DVE drain of the previous chunk.

PSUM budget (8 banks): sct ring 4 | avq 2 (one bank per head) | shared
transpose/O-proj ring 2. Phase 1 swaps the last ring for a 2-bank
projection ring (LIFO pool order). Matmuls: f32r for scores (full rate at
free dim >= 256), bf16 elsewhere; f32 accumulation throughout.
"""

import numpy as np

B, S, D = 2, 2048, 1024
H, DK = 16, 64
NCORES = 8
DS = 256            # model-dim slice per core (4 heads x 64)
P = 128
DKA = DK + 1        # v columns + softmax-denominator ones column

_cache = {}


def _build(repeat=1):
    import concourse.mybir as mybir
    import concourse.tile as tile
    from concourse import bacc

    f32 = mybir.dt.float32
    f32r = mybir.dt.float32r
    bf16 = mybir.dt.bfloat16
    Exp = mybir.ActivationFunctionType.Exp
    add = mybir.AluOpType.add
    mult = mybir.AluOpType.mult

    nc = bacc.Bacc("TRN2", target_bir_lowering=False, debug=False,
                   num_devices=NCORES)

    xq_d = nc.dram_tensor("xq", [S, D], bf16, kind="ExternalInput")
    xk_d = nc.dram_tensor("xk", [S, D], bf16, kind="ExternalInput")
    xv_d = nc.dram_tensor("xv", [S, D], bf16, kind="ExternalInput")
    wqT_d = nc.dram_tensor("wqT", [D, DS], bf16, kind="ExternalInput")
    wkT_d = nc.dram_tensor("wkT", [D, DS], bf16, kind="ExternalInput")
    wvT_d = nc.dram_tensor("wvT", [D, DS], bf16, kind="ExternalInput")
    woT_d = nc.dram_tensor("woT", [DS, D], bf16, kind="ExternalInput")
    bq_d = nc.dram_tensor("bq", [2, P, 1], f32, kind="ExternalInput")
    bk_d = nc.dram_tensor("bk", [2, P, 1], f32, kind="ExternalInput")
    bvr_d = nc.dram_tensor("bvr", [P, DS], bf16, kind="ExternalInput")
    id_d = nc.dram_tensor("ident", [P, P], bf16, kind="ExternalInput")
    y_d = nc.dram_tensor("y", [S, D], f32, kind="ExternalOutput")

    with tile.TileContext(nc) as tc:
        with (
            tc.tile_pool(name="persist", bufs=1) as pp,
            tc.tile_pool(name="xT", bufs=4) as xtp,
            tc.tile_pool(name="pt", bufs=48) as ptp,
            tc.tile_pool(name="attn", bufs=4) as asp,
            tc.tile_pool(name="small", bufs=2) as smp,
            tc.tile_pool(name="ysb", bufs=2) as yp,
        ):
            # ---- constants / weights ----
            wq_bf = pp.tile([P, 8, DS], bf16)
            wk_bf = pp.tile([P, 8, DS], bf16)
            wv_bf = pp.tile([P, 8, DS], bf16)
            wo_bf = pp.tile([P, 2, D], bf16)
            bq_sb = pp.tile([P, 2, 1], f32)
            bk_sb = pp.tile([P, 2, 1], f32)
            bv_sb = pp.tile([P, DS], bf16)
            id_sb = pp.tile([P, P], bf16)

            # ---- persistent activations ----
            QT = pp.tile([P, 2, S], f32r)      # [dk-in-pair, head-pair, token]
            KT = pp.tile([P, 2, S], f32r)
            V = pp.tile([P, 16, 4, DKA], bf16)  # [key-in-tile, kt, head, dk+1]
            attnT = pp.tile([P, 2, S], bf16)   # [dk-in-pair, head-pair, token]
            # softmax-denominator ones column, preset once
            nc.vector.memset(V[:, :, :, DK:DKA], 1.0)

            for _rep in range(repeat):
                sc_ctx = tc.tile_pool(name="sc_ps", bufs=2, space="PSUM")
                scp = sc_ctx.__enter__()
                tr_ctx = tc.tile_pool(name="tr_ps", bufs=2, space="PSUM")
                trp = tr_ctx.__enter__()

                # K-projection inputs first on the DMA queue
                nc.scalar.dma_start(
                    wk_bf[:], wkT_d.ap().rearrange("(c p) d -> p c d", p=P))

                warm = pp.tile([P, 128], bf16, name="warm", tag="warm") \
                    if _rep == 0 else warm
                if _rep == 0:
                    nc.vector.memset(warm[:], 0.0)
                # ~5us of dependency-free matmuls: keeps the PE busy (and
                # its clock-gate warm) through the startup DMA fill, so the
                # first projection matmuls run at full clock
                for _w in range(48):
                    wps = trp.tile([P, 512], f32, tag="pj", name="wps")
                    nc.tensor.matmul(wps[:, 0:128], warm[:], warm[:],
                                     start=True, stop=True)

                pts = {}

                def emit_scores(qb, kt):
                    qs = slice(qb * 512, (qb + 1) * 512)
                    for hp in range(2):
                        sct = scp.tile([P, 2, 512], f32, tag="sct")
                        for hh in range(2):
                            hb = 64 * hh
                            nc.tensor.matmul(
                                sct[:, hh, :],
                                KT[hb:hb + 64, hp, kt * P:(kt + 1) * P],
                                QT[hb:hb + 64, hp, qs],
                                start=True, stop=True)
                        pt = ptp.tile([P, 2, 512], bf16, tag="pt")
                        nc.scalar.activation(pt[:], sct[:], Exp, scale=0.125)
                        pts[(qb, kt, hp)] = pt

                def load_xT(x_d, t4):
                    xT = xtp.tile([P, 8, 512], bf16, tag="xT")
                    for i in range(4):
                        tb = 4 * t4 + i
                        nc.sync.dma_start_transpose(
                            xT[:, :, i * P:(i + 1) * P],
                            x_d.ap()[tb * P:(tb + 1) * P, :])
                    return xT

                def emit_proj(kind, t4, xT, hp, fine=False, pool=None):
                    w = wk_bf if kind == "k" else wq_bf
                    bias = bk_sb if kind == "k" else bq_sb
                    out = KT if kind == "k" else QT
                    # fine=True: 128-token chunks so the first matmuls start
                    # after transpose 0 lands, not after all four
                    chunks = [slice(128 * i, 128 * (i + 1)) for i in range(4)] \
                        if fine else [slice(0, 512)]
                    if pool is None:
                        ps = trp.tile([P, 512], f32, tag="pj", name="ps")
                    else:
                        ps = pool.tile([P, 512], f32, tag="tpy", name="ps")
                    ob = t4 * 512
                    for cs in chunks:
                        for ch in range(8):
                            nc.tensor.matmul(
                                ps[:, cs], w[:, ch, hp * P:(hp + 1) * P],
                                xT[:, ch, cs],
                                start=(ch == 0), stop=(ch == 7))
                        if fine:
                            nc.vector.tensor_scalar(
                                out[:, hp, ob + cs.start:ob + cs.stop],
                                ps[:, cs], bias[:, hp, :], None, op0=add)
                    if not fine:
                        nc.vector.tensor_scalar(
                            out[:, hp, ob:ob + 512], ps[:],
                            bias[:, hp, :], None, op0=add)

                def emit_v_half(xT, t4, half):
                    pv = trp.tile([P, 512], f32, tag="pj")
                    pvv = pv[:].rearrange("p (t d) -> p t d", t=2)
                    for j in range(2):
                        ti = 2 * half + j
                        for ch in range(8):
                            nc.tensor.matmul(
                                pvv[:, j, :],
                                xT[:, ch, ti * P:(ti + 1) * P],
                                wv_bf[:, ch, :],
                                start=(ch == 0), stop=(ch == 7))
                    for j in range(2):
                        tb = 4 * t4 + 2 * half + j
                        nc.vector.tensor_add(
                            V[:, tb, :, 0:DK],
                            pvv[:, j, :].rearrange("p (h d) -> p h d", h=4),
                            bv_sb[:].rearrange("p (h d) -> p h d", h=4))

                # ---- phase 1: K/V/Q0 projections fused with qb0 scores;
                # K is chunk-interleaved so each score tile emits right
                # after its 128-token K chunk and the exp stream never gaps
                xq_tiles = {}
                for t4 in range(4):
                    xTk = load_xT(xk_d, t4)
                    if t4 == 0:
                        xq_tiles[0] = load_xT(xq_d, 0)
                        nc.scalar.dma_start(bk_sb[:, 0, :], bk_d.ap()[0])
                        nc.scalar.dma_start(bk_sb[:, 1, :], bk_d.ap()[1])
                        nc.scalar.dma_start(
                            wq_bf[:],
                            wqT_d.ap().rearrange("(c p) d -> p c d", p=P))
                        nc.scalar.dma_start(bq_sb[:, 0, :], bq_d.ap()[0])
                        nc.scalar.dma_start(bq_sb[:, 1, :], bq_d.ap()[1])
                        emit_proj("k", 0, xTk, 0, fine=True)
                        emit_proj("k", 0, xTk, 1, fine=True)
                        emit_proj("q", 0, xq_tiles[0], 0, fine=True)
                        emit_proj("q", 0, xq_tiles[0], 1, fine=True)
                        # deferred constant loads, off the startup DMA path
                        nc.scalar.dma_start(
                            wv_bf[:],
                            wvT_d.ap().rearrange("(c p) d -> p c d", p=P))
                        nc.scalar.dma_start(bv_sb[:], bvr_d.ap())
                        nc.scalar.dma_start(id_sb[:], id_d.ap())
                        nc.scalar.dma_start(
                            wo_bf[:],
                            woT_d.ap().rearrange("(c p) d -> p c d", p=P))
                        xTv = load_xT(xv_d, t4)
                        emit_scores(0, 0)
                        emit_scores(0, 1)
                        emit_v_half(xTv, t4, 0)
                        emit_scores(0, 2)
                        emit_scores(0, 3)
                        emit_v_half(xTv, t4, 1)
                        continue
                    xTv = load_xT(xv_d, t4)
                    kps = [trp.tile([P, 512], f32, tag="kps", name=f"kps{_h}")
                           for _h in range(2)]
                    for i in range(4):
                        cs = slice(128 * i, 128 * (i + 1))
                        for hp in range(2):
                            for ch in range(8):
                                nc.tensor.matmul(
                                    kps[hp][:, cs],
                                    wk_bf[:, ch, hp * P:(hp + 1) * P],
                                    xTk[:, ch, cs],
                                    start=(ch == 0), stop=(ch == 7))
                            nc.vector.tensor_scalar(
                                KT[:, hp, t4 * 512 + cs.start:
                                   t4 * 512 + cs.stop],
                                kps[hp][:, cs], bk_sb[:, hp, :],
                                None, op0=add)
                        emit_scores(0, 4 * t4 + i)
                        if i == 1:
                            emit_v_half(xTv, t4, 0)
                        if i == 3:
                            emit_v_half(xTv, t4, 1)

                # phase 1 projection PSUM ring -> AV + transpose/O-proj rings
                tr_ctx.__exit__(None, None, None)
                av_ctx = tc.tile_pool(name="av_ps", bufs=2, space="PSUM")
                avp = av_ctx.__enter__()
                ty_ctx = tc.tile_pool(name="ty_ps", bufs=2, space="PSUM")
                typ = ty_ctx.__enter__()

                # ---- attention (hp-outer) + work-queue filler ----
                work = []
                attn_sbs = {}
                y_sbs = {}

                def emit_T(qb, hp):
                    # PE transposes attn [q, dk] -> attnT [dk, q]; 2 heads
                    # stack per instruction via the [q, (hh dk)] input view
                    tpt = typ.tile([P, 4, P], bf16, tag="tpy", name="tpt")
                    a_sb = attn_sbs.pop((qb, hp))
                    for qt in range(4):
                        nc.tensor.transpose(tpt[:, qt, :],
                                            a_sb[:, qt, :, :], id_sb[:])
                    nc.vector.tensor_copy(
                        attnT[:, hp, qb * 512:(qb + 1) * 512]
                        .rearrange("p (t q) -> p t q", t=4),
                        tpt[:, :, :])

                tail = [False]

                def emit_y_half(tt, nb):
                    if tt not in y_sbs:
                        y_sbs[tt] = yp.tile([P, D], f32, name="y_sb", tag="y")
                    y_sb = y_sbs[tt]
                    py = typ.tile([P, 512], f32, tag="tpy", name="py")
                    for hpc in range(2):
                        nc.tensor.matmul(
                            py[:],
                            attnT[:, hpc, tt * P:(tt + 1) * P],
                            wo_bf[:, hpc, nb * 512:(nb + 1) * 512],
                            start=(hpc == 0), stop=(hpc == 1))
                    if tail[0] and (tt + nb) % 2 == 0:
                        nc.scalar.copy(y_sb[:, nb * 512:(nb + 1) * 512],
                                       py[:])
                    else:
                        nc.vector.tensor_copy(
                            y_sb[:, nb * 512:(nb + 1) * 512], py[:])
                    if nb == 1:
                        nc.scalar.dma_start(y_d.ap()[tt * P:(tt + 1) * P, :],
                                          y_sb[:])
                        del y_sbs[tt]

                def emit_work(n):
                    for _ in range(n):
                        if not work:
                            return
                        kind, *a = work.pop(0)
                        if kind == "S":
                            emit_scores(*a)
                        elif kind == "XQ":
                            xq_tiles[a[0]] = load_xT(xq_d, a[0])
                        elif kind == "Q":
                            emit_proj("q", a[0], xq_tiles[a[0]], a[1],
                                      pool=typ)
                        elif kind == "T":
                            emit_T(*a)
                        else:
                            emit_y_half(*a)

                xq_tiles[1] = load_xT(xq_d, 1)
                for qb in range(4):
                    if qb < 3:
                        # next-qb Q projection + scores feed the queue;
                        # ordering keeps Q(t4) ahead of its S(qb, *) users
                        if qb == 0:
                            work.append(("Q", 1, 0))
                            work.append(("Q", 1, 1))
                            work.append(("XQ", 2))
                        for kt in range(16):
                            work.append(("S", qb + 1, kt))
                        if qb == 0:
                            work.append(("XQ", 3))
                            work.append(("Q", 2, 0))
                            work.append(("Q", 2, 1))
                        if qb == 1:
                            work.append(("Q", 3, 0))
                            work.append(("Q", 3, 1))
                    for hp in range(2):
                        avq = [avp.tile([P, 4, DKA], f32, tag="avq",
                                        name=f"avq{_h}")
                               for _h in range(2)]
                        for kt in range(16):
                            # PE queue is in-order: pop independent filler
                            # BEFORE the exp-gated AV matmuls so it runs
                            # during the wait
                            if kt % 2 == 0:
                                emit_work(1)
                            if (hp * 16 + kt) % 4 == 0:
                                emit_work(1)
                            ptm = pts[(qb, kt, hp)]
                            for hh in range(2):
                                h = 2 * hp + hh
                                for qt in range(4):
                                    # one PSUM accumulation group per bank:
                                    # start marks the whole 2KB zero region,
                                    # each qt's first write then zero-fills
                                    nc.tensor.matmul(
                                        avq[hh][:, qt, :],
                                        ptm[:, hh, qt * P:(qt + 1) * P],
                                        V[:, kt, h, :],
                                        start=(kt == 0 and qt == 0),
                                        stop=(kt == 15 and qt == 3))
                            del pts[(qb, kt, hp)]
                        # boundary filler between the final AV and the DVE
                        # normalization chain
                        emit_work(2)
                        # partition-aligned normalization: reciprocal of the
                        # accumulated denominator column, then one
                        # tensor_scalar multiply per [128, 64] block
                        a_sb = asp.tile([P, 4, 2, DK], bf16, tag="attn")
                        attn_sbs[(qb, hp)] = a_sb
                        for hh in range(2):
                            rec = smp.tile([P, 4, 1], f32, tag="rec")
                            nc.vector.reciprocal(rec[:],
                                                 avq[hh][:, :, DK:DKA])
                            for qt in range(4):
                                nc.vector.tensor_scalar(
                                    a_sb[:, qt, hh, :],
                                    avq[hh][:, qt, 0:DK],
                                    rec[:, qt, :], None, op0=mult)
                        work.append(("T", qb, hp))
                        if hp == 1:
                            for tt in range(4 * qb, 4 * qb + 4):
                                for nb in range(2):
                                    work.append(("Y", tt, nb))
                tail[0] = True
                emit_work(len(work))

                for ctx in (ty_ctx, av_ctx, sc_ctx):
                    ctx.__exit__(None, None, None)

    nc.compile()
    return nc


def _shard(query, key, value, Wq, bq, Wk, bk, Wv, bv, Wo, bo):
    import ml_dtypes
    f = np.float32
    bf = ml_dtypes.bfloat16
    q = np.asarray(query, dtype=f).astype(bf)
    k = np.asarray(key, dtype=f).astype(bf)
    v = np.asarray(value, dtype=f).astype(bf)
    ident = np.eye(P, dtype=bf)
    in_maps = []
    for c in range(NCORES):
        b, hg = c // 4, c % 4
        ds = DS * hg
        bv_r = np.broadcast_to(
            np.asarray(bv, f)[ds:ds + DS].astype(bf)[None, :], (P, DS))
        in_maps.append({
            "xq": np.ascontiguousarray(q[b]),
            "xk": np.ascontiguousarray(k[b]),
            "xv": np.ascontiguousarray(v[b]),
            "wqT": np.ascontiguousarray(np.asarray(Wq, f)[ds:ds + DS, :].T.astype(bf)),
            "wkT": np.ascontiguousarray(np.asarray(Wk, f)[ds:ds + DS, :].T.astype(bf)),
            "wvT": np.ascontiguousarray(np.asarray(Wv, f)[ds:ds + DS, :].T.astype(bf)),
            "woT": np.ascontiguousarray(np.asarray(Wo, f)[:, ds:ds + DS].T.astype(bf)),
            "bq": np.asarray(bq, f)[ds:ds + DS].reshape(2, P, 1),
            "bk": np.asarray(bk, f)[ds:ds + DS].reshape(2, P, 1),
            "bvr": np.ascontiguousarray(bv_r),
            "ident": ident,
        })
    return in_maps


def _unshard(results, bo):
    y = np.zeros((B, S, D), dtype=np.float64)
    for c in range(NCORES):
        y[c // 4] += results[c]["y"].astype(np.float64)
    y += np.asarray(bo, np.float64)
    return y.astype(np.float32)


def kernel(query, key, value, Wq, bq, Wk, bk, Wv, bv, Wo, bo):
    from concourse.bass_utils import run_bass_kernel_spmd

    if "nc" not in _cache:
        _cache["nc"] = _build()
    nc = _cache["nc"]
    in_maps = _shard(query, key, value, Wq, bq, Wk, bk, Wv, bv, Wo, bo)
    res = run_bass_kernel_spmd(nc, in_maps, core_ids=list(range(NCORES)))
    return _unshard(res.results, bo)


# revision 14
# speedup vs baseline: 1.1755x; 1.1748x over previous
"""Multi-head attention (B=2, S=2048, D=1024, H=16, d_k=64) on 8 TRN2 NeuronCores.

Sharding: batch x head-groups. Core c handles batch b = c // 4 and heads
[4*(c%4), 4*(c%4)+4), i.e. a 256-wide slice of the model dim. Host sums the
4 partial y's per batch and adds bo.

Per-core kernel, organized so every matmul instruction's cost (= its
moving/free size) does useful work, and so the Scalar engine (softmax exp,
~133us busy, the co-critical engine next to the PE's ~146us) starts early
and never starves:
  - scores S^T = K Q^T per head-pair: two (64x128)-stationary matmuls per
    kt tile, free dim 512 (f32r, full rate),
  - attention out in [query, d_k] orientation: stationary = exp-score tile
    pt [128 keys, 128 queries], moving = ones-AUGMENTED V [128 keys, 65]
    (64 v-columns + a ones column), accumulated over the 16 key tiles in
    PSUM (one accumulation group per bank: start only on the first write,
    stop on the last - PSUM zero regions are 2KB). Free size is 65 instead
    of 512, and column 64 accumulates the softmax denominator for free,
  - normalization is partition-aligned: DVE reciprocal of the denominator
    column + tensor_scalar multiply per [128, 64] block -> attn [q, dk]
    bf16 in SBUF,
  - a PE transpose (identity matmul, 128 rows each) flips attn back to
    [dk, token] for the O-projection; 2 heads per transpose,
  - phase 1 fuses K/V/Q(t4=0) projections with qb0's full score+exp chain
    (kept lean so the first exp lands ~10us in); Q(t4=1..3) projections,
    next-qb scores, transposes and O-projection chunks all flow through one
    ordered work queue, popped as filler BEFORE the exp-gated AV matmuls
    (PE queue is in-order, so filler must precede the stall),
  - transpose + O-proj PSUM tiles share one 2-deep tag ring so consecutive
    O-proj chunks don't serialize on the DVE drain of the previous chunk.

PSUM budget (8 banks): sct ring 4 | avq 2 (one bank per head) | shared
transpose/O-proj ring 2. Phase 1 swaps the last ring for a 2-bank
projection ring (LIFO pool order). Matmuls: f32r for scores (full rate at
free dim >= 256), bf16 elsewhere; f32 accumulation throughout.
"""

import numpy as np

B, S, D = 2, 2048, 1024
H, DK = 16, 64
NCORES = 8
DS = 256            # model-dim slice per core (4 heads x 64)
P = 128
DKA = DK + 1        # v columns + softmax-denominator ones column

_cache = {}


def _build(repeat=1):
    import concourse.mybir as mybir
    import concourse.tile as tile
    from concourse import bacc

    f32 = mybir.dt.float32
    f32r = mybir.dt.float32r
    bf16 = mybir.dt.bfloat16
    Exp = mybir.ActivationFunctionType.Exp
    add = mybir.AluOpType.add
    mult = mybir.AluOpType.mult

    nc = bacc.Bacc("TRN2", target_bir_lowering=False, debug=False,
                   num_devices=NCORES)

    xq_d = nc.dram_tensor("xq", [D, S], bf16, kind="ExternalInput")
    xk_d = nc.dram_tensor("xk", [D, S], bf16, kind="ExternalInput")
    xv_d = nc.dram_tensor("xv", [D, S], bf16, kind="ExternalInput")
    wqT_d = nc.dram_tensor("wqT", [D, DS], bf16, kind="ExternalInput")
    wkT_d = nc.dram_tensor("wkT", [D, DS], bf16, kind="ExternalInput")
    wvT_d = nc.dram_tensor("wvT", [D, DS], bf16, kind="ExternalInput")
    woT_d = nc.dram_tensor("woT", [DS, D], bf16, kind="ExternalInput")
    bq_d = nc.dram_tensor("bq", [2, P, 1], f32, kind="ExternalInput")
    bk_d = nc.dram_tensor("bk", [2, P, 1], f32, kind="ExternalInput")
    bvr_d = nc.dram_tensor("bvr", [P, DS], bf16, kind="ExternalInput")
    id_d = nc.dram_tensor("ident", [P, P], bf16, kind="ExternalInput")
    y_d = nc.dram_tensor("y", [S, D], f32, kind="ExternalOutput")

    with tile.TileContext(nc) as tc:
        with (
            tc.tile_pool(name="persist", bufs=1) as pp,
            tc.tile_pool(name="xT", bufs=4) as xtp,
            tc.tile_pool(name="pt", bufs=48) as ptp,
            tc.tile_pool(name="attn", bufs=4) as asp,
            tc.tile_pool(name="small", bufs=2) as smp,
            tc.tile_pool(name="ysb", bufs=2) as yp,
        ):
            # ---- constants / weights ----
            wq_bf = pp.tile([P, 8, DS], bf16)
            wk_bf = pp.tile([P, 8, DS], bf16)
            wv_bf = pp.tile([P, 8, DS], bf16)
            wo_bf = pp.tile([P, 2, D], bf16)
            bq_sb = pp.tile([P, 2, 1], f32)
            bk_sb = pp.tile([P, 2, 1], f32)
            bv_sb = pp.tile([P, DS], bf16)
            id_sb = pp.tile([P, P], bf16)

            # ---- persistent activations ----
            QT = pp.tile([P, 2, S], f32r)      # [dk-in-pair, head-pair, token]
            KT = pp.tile([P, 2, S], f32r)
            V = pp.tile([P, 16, 4, DKA], bf16)  # [key-in-tile, kt, head, dk+1]
            attnT = pp.tile([P, 2, S], bf16)   # [dk-in-pair, head-pair, token]
            # softmax-denominator ones column, preset once
            nc.vector.memset(V[:, :, :, DK:DKA], 1.0)

            for _rep in range(repeat):
                sc_ctx = tc.tile_pool(name="sc_ps", bufs=2, space="PSUM")
                scp = sc_ctx.__enter__()
                tr_ctx = tc.tile_pool(name="tr_ps", bufs=2, space="PSUM")
                trp = tr_ctx.__enter__()

                # K-projection inputs first on the DMA queue
                nc.scalar.dma_start(
                    wk_bf[:], wkT_d.ap().rearrange("(c p) d -> p c d", p=P))

                warm = pp.tile([P, 128], bf16, name="warm", tag="warm") \
                    if _rep == 0 else warm
                if _rep == 0:
                    nc.vector.memset(warm[:], 0.0)
                # ~5us of dependency-free matmuls: keeps the PE busy (and
                # its clock-gate warm) through the startup DMA fill, so the
                # first projection matmuls run at full clock
                for _w in range(48):
                    wps = trp.tile([P, 512], f32, tag="pj", name="wps")
                    nc.tensor.matmul(wps[:, 0:128], warm[:], warm[:],
                                     start=True, stop=True)

                pts = {}

                def emit_scores(qb, kt):
                    qs = slice(qb * 512, (qb + 1) * 512)
                    for hp in range(2):
                        sct = scp.tile([P, 2, 512], f32, tag="sct")
                        for hh in range(2):
                            hb = 64 * hh
                            nc.tensor.matmul(
                                sct[:, hh, :],
                                KT[hb:hb + 64, hp, kt * P:(kt + 1) * P],
                                QT[hb:hb + 64, hp, qs],
                                start=True, stop=True)
                        pt = ptp.tile([P, 2, 512], bf16, tag="pt")
                        nc.scalar.activation(pt[:], sct[:], Exp, scale=0.125)
                        pts[(qb, kt, hp)] = pt

                def load_xT(x_d, t4):
                    # x comes host-transposed [D, S]: plain contiguous DMA
                    xT = xtp.tile([P, 8, 512], bf16, tag="xT")
                    nc.sync.dma_start(
                        xT[:],
                        x_d.ap().rearrange("(c p) (f t) -> p c f t",
                                           p=P, f=4)[:, :, t4, :])
                    return xT

                def emit_proj(kind, t4, xT, hp, fine=False, pool=None):
                    w = wk_bf if kind == "k" else wq_bf
                    bias = bk_sb if kind == "k" else bq_sb
                    out = KT if kind == "k" else QT
                    # fine=True: 128-token chunks so the first matmuls start
                    # after transpose 0 lands, not after all four
                    chunks = [slice(128 * i, 128 * (i + 1)) for i in range(4)] \
                        if fine else [slice(0, 512)]
                    if pool is None:
                        ps = trp.tile([P, 512], f32, tag="pj", name="ps")
                    else:
                        ps = pool.tile([P, 512], f32, tag="tpy", name="ps")
                    ob = t4 * 512
                    for cs in chunks:
                        for ch in range(8):
                            nc.tensor.matmul(
                                ps[:, cs], w[:, ch, hp * P:(hp + 1) * P],
                                xT[:, ch, cs],
                                start=(ch == 0), stop=(ch == 7))
                        if fine:
                            nc.vector.tensor_scalar(
                                out[:, hp, ob + cs.start:ob + cs.stop],
                                ps[:, cs], bias[:, hp, :], None, op0=add)
                    if not fine:
                        nc.vector.tensor_scalar(
                            out[:, hp, ob:ob + 512], ps[:],
                            bias[:, hp, :], None, op0=add)

                def emit_v_half(xT, t4, half):
                    pv = trp.tile([P, 512], f32, tag="pj")
                    pvv = pv[:].rearrange("p (t d) -> p t d", t=2)
                    for j in range(2):
                        ti = 2 * half + j
                        for ch in range(8):
                            nc.tensor.matmul(
                                pvv[:, j, :],
                                xT[:, ch, ti * P:(ti + 1) * P],
                                wv_bf[:, ch, :],
                                start=(ch == 0), stop=(ch == 7))
                    for j in range(2):
                        tb = 4 * t4 + 2 * half + j
                        nc.vector.tensor_add(
                            V[:, tb, :, 0:DK],
                            pvv[:, j, :].rearrange("p (h d) -> p h d", h=4),
                            bv_sb[:].rearrange("p (h d) -> p h d", h=4))

                # ---- phase 1: K/V/Q0 projections fused with qb0 scores;
                # K is chunk-interleaved so each score tile emits right
                # after its 128-token K chunk and the exp stream never gaps
                xq_tiles = {}
                for t4 in range(4):
                    xTk = load_xT(xk_d, t4)
                    if t4 == 0:
                        xq_tiles[0] = load_xT(xq_d, 0)
                        nc.scalar.dma_start(bk_sb[:, 0, :], bk_d.ap()[0])
                        nc.scalar.dma_start(bk_sb[:, 1, :], bk_d.ap()[1])
                        nc.scalar.dma_start(
                            wq_bf[:],
                            wqT_d.ap().rearrange("(c p) d -> p c d", p=P))
                        nc.scalar.dma_start(bq_sb[:, 0, :], bq_d.ap()[0])
                        nc.scalar.dma_start(bq_sb[:, 1, :], bq_d.ap()[1])
                        emit_proj("k", 0, xTk, 0, fine=True)
                        emit_proj("k", 0, xTk, 1, fine=True)
                        emit_proj("q", 0, xq_tiles[0], 0, fine=True)
                        emit_proj("q", 0, xq_tiles[0], 1, fine=True)
                        # deferred constant loads, off the startup DMA path
                        nc.scalar.dma_start(
                            wv_bf[:],
                            wvT_d.ap().rearrange("(c p) d -> p c d", p=P))
                        nc.scalar.dma_start(bv_sb[:], bvr_d.ap())
                        nc.scalar.dma_start(id_sb[:], id_d.ap())
                        nc.scalar.dma_start(
                            wo_bf[:],
                            woT_d.ap().rearrange("(c p) d -> p c d", p=P))
                        xTv = load_xT(xv_d, t4)
                        emit_scores(0, 0)
                        emit_scores(0, 1)
                        emit_v_half(xTv, t4, 0)
                        emit_scores(0, 2)
                        emit_scores(0, 3)
                        emit_v_half(xTv, t4, 1)
                        continue
                    xTv = load_xT(xv_d, t4)
                    kps = [trp.tile([P, 512], f32, tag="kps", name=f"kps{_h}")
                           for _h in range(2)]
                    for i in range(4):
                        cs = slice(128 * i, 128 * (i + 1))
                        for hp in range(2):
                            for ch in range(8):
                                nc.tensor.matmul(
                                    kps[hp][:, cs],
                                    wk_bf[:, ch, hp * P:(hp + 1) * P],
                                    xTk[:, ch, cs],
                                    start=(ch == 0), stop=(ch == 7))
                            nc.vector.tensor_scalar(
                                KT[:, hp, t4 * 512 + cs.start:
                                   t4 * 512 + cs.stop],
                                kps[hp][:, cs], bk_sb[:, hp, :],
                                None, op0=add)
                        emit_scores(0, 4 * t4 + i)
                        if i == 1:
                            emit_v_half(xTv, t4, 0)
                        if i == 3:
                            emit_v_half(xTv, t4, 1)

                # phase 1 projection PSUM ring -> AV + transpose/O-proj rings
                tr_ctx.__exit__(None, None, None)
                av_ctx = tc.tile_pool(name="av_ps", bufs=2, space="PSUM")
                avp = av_ctx.__enter__()
                ty_ctx = tc.tile_pool(name="ty_ps", bufs=2, space="PSUM")
                typ = ty_ctx.__enter__()

                # ---- attention (hp-outer) + work-queue filler ----
                work = []
                attn_sbs = {}
                y_sbs = {}

                def emit_T(qb, hp):
                    # PE transposes attn [q, dk] -> attnT [dk, q]; 2 heads
                    # stack per instruction via the [q, (hh dk)] input view
                    tpt = typ.tile([P, 4, P], bf16, tag="tpy", name="tpt")
                    a_sb = attn_sbs.pop((qb, hp))
                    for qt in range(4):
                        nc.tensor.transpose(tpt[:, qt, :],
                                            a_sb[:, qt, :, :], id_sb[:])
                    nc.vector.tensor_copy(
                        attnT[:, hp, qb * 512:(qb + 1) * 512]
                        .rearrange("p (t q) -> p t q", t=4),
                        tpt[:, :, :])

                tail = [False]

                def emit_y_half(tt, nb):
                    if tt not in y_sbs:
                        y_sbs[tt] = yp.tile([P, D], f32, name="y_sb", tag="y")
                    y_sb = y_sbs[tt]
                    py = typ.tile([P, 512], f32, tag="tpy", name="py")
                    for hpc in range(2):
                        nc.tensor.matmul(
                            py[:],
                            attnT[:, hpc, tt * P:(tt + 1) * P],
                            wo_bf[:, hpc, nb * 512:(nb + 1) * 512],
                            start=(hpc == 0), stop=(hpc == 1))
                    if tail[0] and (tt + nb) % 2 == 0:
                        nc.scalar.copy(y_sb[:, nb * 512:(nb + 1) * 512],
                                       py[:])
                    else:
                        nc.vector.tensor_copy(
                            y_sb[:, nb * 512:(nb + 1) * 512], py[:])
                    if nb == 1:
                        nc.scalar.dma_start(y_d.ap()[tt * P:(tt + 1) * P, :],
                                          y_sb[:])
                        del y_sbs[tt]

                def emit_work(n):
                    for _ in range(n):
                        if not work:
                            return
                        kind, *a = work.pop(0)
                        if kind == "S":
                            emit_scores(*a)
                        elif kind == "XQ":
                            xq_tiles[a[0]] = load_xT(xq_d, a[0])
                        elif kind == "Q":
                            emit_proj("q", a[0], xq_tiles[a[0]], a[1],
                                      pool=typ)
                        elif kind == "T":
                            emit_T(*a)
                        else:
                            emit_y_half(*a)

                xq_tiles[1] = load_xT(xq_d, 1)
                for qb in range(4):
                    if qb < 3:
                        # next-qb Q projection + scores feed the queue;
                        # ordering keeps Q(t4) ahead of its S(qb, *) users
                        if qb == 0:
                            work.append(("Q", 1, 0))
                            work.append(("Q", 1, 1))
                            work.append(("XQ", 2))
                        for kt in range(16):
                            work.append(("S", qb + 1, kt))
                        if qb == 0:
                            work.append(("XQ", 3))
                            work.append(("Q", 2, 0))
                            work.append(("Q", 2, 1))
                        if qb == 1:
                            work.append(("Q", 3, 0))
                            work.append(("Q", 3, 1))
                    for hp in range(2):
                        avq = [avp.tile([P, 4, DKA], f32, tag="avq",
                                        name=f"avq{_h}")
                               for _h in range(2)]
                        for kt in range(16):
                            # PE queue is in-order: pop independent filler
                            # BEFORE the exp-gated AV matmuls so it runs
                            # during the wait
                            if kt % 2 == 0:
                                emit_work(1)
                            if (hp * 16 + kt) % 4 == 0:
                                emit_work(1)
                            ptm = pts[(qb, kt, hp)]
                            for hh in range(2):
                                h = 2 * hp + hh
                                for qt in range(4):
                                    # one PSUM accumulation group per bank:
                                    # start marks the whole 2KB zero region,
                                    # each qt's first write then zero-fills
                                    nc.tensor.matmul(
                                        avq[hh][:, qt, :],
                                        ptm[:, hh, qt * P:(qt + 1) * P],
                                        V[:, kt, h, :],
                                        start=(kt == 0 and qt == 0),
                                        stop=(kt == 15 and qt == 3))
                            del pts[(qb, kt, hp)]
                        # boundary filler between the final AV and the DVE
                        # normalization chain
                        emit_work(2)
                        # partition-aligned normalization: reciprocal of the
                        # accumulated denominator column, then one
                        # tensor_scalar multiply per [128, 64] block
                        a_sb = asp.tile([P, 4, 2, DK], bf16, tag="attn")
                        attn_sbs[(qb, hp)] = a_sb
                        for hh in range(2):
                            rec = smp.tile([P, 4, 1], f32, tag="rec")
                            nc.vector.reciprocal(rec[:],
                                                 avq[hh][:, :, DK:DKA])
                            for qt in range(4):
                                nc.vector.tensor_scalar(
                                    a_sb[:, qt, hh, :],
                                    avq[hh][:, qt, 0:DK],
                                    rec[:, qt, :], None, op0=mult)
                        work.append(("T", qb, hp))
                        if hp == 1:
                            for tt in range(4 * qb, 4 * qb + 4):
                                for nb in range(2):
                                    work.append(("Y", tt, nb))
                tail[0] = True
                emit_work(len(work))

                for ctx in (ty_ctx, av_ctx, sc_ctx):
                    ctx.__exit__(None, None, None)

    nc.compile()
    return nc


def _shard(query, key, value, Wq, bq, Wk, bk, Wv, bv, Wo, bo):
    import ml_dtypes
    f = np.float32
    bf = ml_dtypes.bfloat16
    q = np.asarray(query, dtype=f).astype(bf)
    k = np.asarray(key, dtype=f).astype(bf)
    v = np.asarray(value, dtype=f).astype(bf)
    qT = [np.ascontiguousarray(q[b].T) for b in range(B)]
    kT = [np.ascontiguousarray(k[b].T) for b in range(B)]
    vT = [np.ascontiguousarray(v[b].T) for b in range(B)]
    ident = np.eye(P, dtype=bf)
    in_maps = []
    for c in range(NCORES):
        b, hg = c // 4, c % 4
        ds = DS * hg
        bv_r = np.broadcast_to(
            np.asarray(bv, f)[ds:ds + DS].astype(bf)[None, :], (P, DS))
        in_maps.append({
            "xq": qT[b],
            "xk": kT[b],
            "xv": vT[b],
            "wqT": np.ascontiguousarray(np.asarray(Wq, f)[ds:ds + DS, :].T.astype(bf)),
            "wkT": np.ascontiguousarray(np.asarray(Wk, f)[ds:ds + DS, :].T.astype(bf)),
            "wvT": np.ascontiguousarray(np.asarray(Wv, f)[ds:ds + DS, :].T.astype(bf)),
            "woT": np.ascontiguousarray(np.asarray(Wo, f)[:, ds:ds + DS].T.astype(bf)),
            "bq": np.asarray(bq, f)[ds:ds + DS].reshape(2, P, 1),
            "bk": np.asarray(bk, f)[ds:ds + DS].reshape(2, P, 1),
            "bvr": np.ascontiguousarray(bv_r),
            "ident": ident,
        })
    return in_maps


def _unshard(results, bo):
    y = np.zeros((B, S, D), dtype=np.float64)
    for c in range(NCORES):
        y[c // 4] += results[c]["y"].astype(np.float64)
    y += np.asarray(bo, np.float64)
    return y.astype(np.float32)


def kernel(query, key, value, Wq, bq, Wk, bk, Wv, bv, Wo, bo):
    from concourse.bass_utils import run_bass_kernel_spmd

    if "nc" not in _cache:
        _cache["nc"] = _build()
    nc = _cache["nc"]
    in_maps = _shard(query, key, value, Wq, bq, Wk, bk, Wv, bv, Wo, bo)
    res = run_bass_kernel_spmd(nc, in_maps, core_ids=list(range(NCORES)))
    return _unshard(res.results, bo)
